# revision 1
# baseline (speedup 1.0000x reference)
"""3-layer GCN (PyG GCNConv x3 + FC) on 8 Trainium2 NeuronCores.

Self-contained: host-side graph packing + Bass kernel + PJRT SPMD runner.

Sharding: core r owns destination nodes [r*12500, (r+1)*12500) and all their
incoming edges (plus self loops). Weights replicated. Per layer, each core
gathers source rows from a replicated feature table (AllGather between
layers), reduces per-destination via two matmul stages, applies bias +
leaky-relu, and pre-applies the next layer's weight matrix before the
AllGather.

All 8 cores execute ONE shared instruction stream; per-core variation is
entirely in streamed data:
  - gather index stream (int16, per-32768-row chunk of the table)
  - "segmap" per 128-edge block: [128 slots, 16] fp32 holding the GCN edge
    norm at (slot, block-local dst column)  -> stage-1 matmul
        pT[16 packed, 64 feat] = segmap.T @ gathered_msgs
    writes a disjoint 16-row stripe of a packed, transposed PSUM tile
  - "S" merge matrices [128 packed, 128 dst] of 0/1 -> stage-2 matmul
    accumulates packed columns into node-ordered window tiles [64, 512].
"""

import sys

sys.path.insert(0, "/opt/trn_rl_repo")

import numpy as np

N_NODES = 100000
N_EDGES = 3200000
IN_F, HID, N_CLS = 10, 64, 10
NEG_SLOPE = 0.01
N_CORES = 8
NS = N_NODES // N_CORES  # 12500 dst nodes per core
CHUNK = 32768  # table rows per gather chunk (int16 index range)
N_CHUNKS = (N_NODES + CHUNK - 1) // CHUNK  # 4
WIN = 512  # dst nodes per window (one PSUM bank of fp32)
N_WIN = (NS + WIN - 1) // WIN  # 25
BLK = 128  # edges per block (PE contraction dim)
STRIPE = 32  # dst columns representable per block
PT_BLKS = 4  # blocks per packed-transpose PSUM tile (32-row PE col groups)
CALL_BLKS = 8  # blocks per dma_gather call (1024 idx: SWDGE ring limit ~2K)
SUBW = 128  # dst per merge subwindow


# ---------------------------------------------------------------------------
# Host-side packing
# ---------------------------------------------------------------------------


def build_plan(edge_index):
    src = np.asarray(edge_index[0], dtype=np.int64)
    dst = np.asarray(edge_index[1], dtype=np.int64)
    deg = np.bincount(dst, minlength=N_NODES).astype(np.float64) + 1.0
    dinv = 1.0 / np.sqrt(deg)
    loop = np.arange(N_NODES, dtype=np.int64)
    s_all = np.concatenate([src, loop])
    d_all = np.concatenate([dst, loop])
    w_all = (dinv[s_all] * dinv[d_all]).astype(np.float32)

    # ---- per-core block packing -------------------------------------------
    # core -> window -> chunk -> list of blocks; each block: (slots' idx,
    # slots' col, slots' norm, block col->dst map)
    cores = []
    for r in range(N_CORES):
        sel = (d_all >= r * NS) & (d_all < (r + 1) * NS)
        es, ed, ew = s_all[sel], d_all[sel] - r * NS, w_all[sel]
        w_id = ed // WIN
        c_id = es // CHUNK
        order = np.lexsort((ed, c_id, w_id))
        es, ed, ew, w_id, c_id = (
            es[order],
            ed[order],
            ew[order],
            w_id[order],
            c_id[order],
        )
        key = w_id * N_CHUNKS + c_id
        run_starts = np.searchsorted(key, np.arange(N_WIN * N_CHUNKS), "left")
        run_ends = np.searchsorted(key, np.arange(N_WIN * N_CHUNKS), "right")

        # dense rank of dst within each run + first-occurrence index
        blocks = {}  # (w, c) -> list of (start, end, rank_at_start)
        n_e = len(es)
        newd = np.empty(n_e, dtype=bool)
        newd[0] = True
        newd[1:] = (ed[1:] != ed[:-1]) | (key[1:] != key[:-1])
        rank = np.cumsum(newd) - 1  # global rank, reset handled via diffs
        first_occ = np.flatnonzero(newd)  # index by rank
        for w in range(N_WIN):
            for c in range(N_CHUNKS):
                a, b = run_starts[w * N_CHUNKS + c], run_ends[w * N_CHUNKS + c]
                lst = []
                p = a
                while p < b:
                    r0 = rank[p]
                    # block may extend until 17th distinct dst or 128 edges
                    lim_rank = r0 + STRIPE
                    lim = first_occ[lim_rank] if lim_rank <= rank[b - 1] else b
                    q = min(p + BLK, lim, b)
                    lst.append((p, q, r0))
                    p = q
                blocks[(w, c)] = lst
        cores.append(
            dict(es=es, ed=ed, ew=ew, rank=rank, blocks=blocks)
        )

    # ---- global uniform structure -----------------------------------------
    b_max = np.zeros((N_WIN, N_CHUNKS), dtype=np.int64)
    for r in range(N_CORES):
        for w in range(N_WIN):
            for c in range(N_CHUNKS):
                b_max[w, c] = max(b_max[w, c], len(cores[r]["blocks"][(w, c)]))
    # pad each window's total block count to a multiple of PT_BLKS so the
    # packed-transpose PSUM tiles [128, 64] are always fully written
    for w in range(N_WIN):
        extra = (-b_max[w].sum()) % PT_BLKS
        b_max[w, N_CHUNKS - 1] += extra

    p0 = np.zeros((N_WIN, N_CHUNKS), dtype=np.int64)  # padded block prefix
    acc = 0
    pw0 = np.zeros(N_WIN + 1, dtype=np.int64)
    for w in range(N_WIN):
        pw0[w] = acc
        for c in range(N_CHUNKS):
            p0[w, c] = acc
            acc += b_max[w, c]
    pw0[N_WIN] = acc
    nblk_tot = acc
    pcw = (pw0[1:] - pw0[:-1]) * STRIPE  # packed cols per window

    # window sizes / merge geometry
    n_w = [min(WIN, NS - w * WIN) for w in range(N_WIN)]
    nslab = [(pcw[w] + 127) // 128 for w in range(N_WIN)]
    nsw = [(n_w[w] + SUBW - 1) // SUBW for w in range(N_WIN)]

    # ---- stream arrays -----------------------------------------------------
    t_idx = nblk_tot * BLK
    idx_streams = np.zeros((N_CORES, 16, t_idx // 16), dtype=np.int16)
    seg_streams = np.zeros((N_CORES, 128, nblk_tot * STRIPE), dtype=np.float32)
    pair_sets = [set() for _ in range(N_WIN)]
    pc2dst_all = []  # [core][w] -> array pcw[w]

    for r in range(N_CORES):
        co = cores[r]
        es, ew, ed, rank = co["es"], co["ew"], co["ed"], co["rank"]
        idx_flat = np.zeros(t_idx, dtype=np.int16)
        pc2dst_w = [np.full(pcw[w], -1, dtype=np.int64) for w in range(N_WIN)]
        for w in range(N_WIN):
            for c in range(N_CHUNKS):
                for j, (a, b, r0) in enumerate(co["blocks"][(w, c)]):
                    g = p0[w, c] + j
                    jw = g - pw0[w]
                    sl = slice(a, b)
                    slot = np.arange(b - a)
                    col = rank[sl] - r0
                    idx_flat[g * BLK + slot] = (es[sl] - c * CHUNK).astype(
                        np.int16
                    )
                    seg_streams[r][slot, g * STRIPE + col] = ew[sl]
                    # packed col -> window-local dst
                    pcs = jw * STRIPE + col
                    pc2dst_w[w][pcs] = ed[sl] - w * WIN
        idx_streams[r] = idx_flat.reshape(-1, 16).T
        pc2dst_all.append(pc2dst_w)
        for w in range(N_WIN):
            pc = np.flatnonzero(pc2dst_w[w] >= 0)
            if len(pc):
                for s, sw in set(
                    zip(pc // 128, pc2dst_w[w][pc] // SUBW)
                ):
                    pair_sets[w].add((int(s), int(sw)))

    # ordered pair list per window: by (sw, s) so each subwindow's
    # accumulation run is contiguous (clean start/stop flags)
    pairs = []  # per window: list of (s, sw, start, stop)
    pair_base = np.zeros(N_WIN + 1, dtype=np.int64)
    tot_pairs = 0
    for w in range(N_WIN):
        ordered = sorted(pair_sets[w], key=lambda t: (t[1], t[0]))
        lst = []
        for i, (s, sw) in enumerate(ordered):
            start = i == 0 or ordered[i - 1][1] != sw
            stop = i == len(ordered) - 1 or ordered[i + 1][1] != sw
            lst.append((s, sw, start, stop))
        pairs.append(lst)
        pair_base[w] = tot_pairs
        tot_pairs += len(lst)
    pair_base[N_WIN] = tot_pairs

    s_streams = np.zeros((N_CORES, 128, tot_pairs * SUBW), dtype=np.float32)
    for r in range(N_CORES):
        for w in range(N_WIN):
            p2d = pc2dst_all[r][w]
            for pi, (s, sw, _a, _b) in enumerate(pairs[w]):
                gp = pair_base[w] + pi
                i0 = s * 128
                rows = np.arange(i0, min(i0 + 128, pcw[w]))
                dloc = p2d[rows]
                m = (dloc >= 0) & (dloc // SUBW == sw)
                s_streams[r][rows[m] - i0, gp * SUBW + (dloc[m] % SUBW)] = 1.0

    idx_full = np.ascontiguousarray(
        np.tile(idx_streams, (1, 8, 1))
    )  # [cores, 128, t_idx//16]

    return dict(
        b_max=b_max,
        p0=p0,
        pw0=pw0,
        pcw=pcw,
        n_w=n_w,
        nslab=nslab,
        nsw=nsw,
        pairs=pairs,
        pair_base=pair_base,
        nblk_tot=int(nblk_tot),
        t_idx=int(t_idx),
        tot_pairs=int(tot_pairs),
        idx=idx_full,
        seg=seg_streams,
        smat=s_streams,
    )


# ---------------------------------------------------------------------------
# Kernel builder
# ---------------------------------------------------------------------------


def build_nc(plan, reps=1, sim_mode=False):
    import concourse.bacc as bacc
    from concourse import mybir
    from concourse.tile import TileContext

    dt = mybir.dt.float32
    b_max = plan["b_max"]
    p0 = plan["p0"]
    pw0 = plan["pw0"]
    pairs = plan["pairs"]
    pair_base = plan["pair_base"]
    n_w = plan["n_w"]

    nc = bacc.Bacc("TRN2", num_devices=1 if sim_mode else N_CORES)

    def allgather(t_loc, t_full):
        if sim_mode:
            # timing stand-in: copy the local shard into its full-table slice
            nc.sync.dma_start(out=t_full[0:NS, :], in_=t_loc[:])
        else:
            nc.gpsimd.collective_compute(
                "AllGather",
                mybir.AluOpType.bypass,
                replica_groups=[list(range(N_CORES))],
                ins=[t_loc[:]],
                outs=[t_full[:]],
            )

    x_pad = nc.dram_tensor("x_pad", [N_NODES, HID], dt, kind="ExternalInput")
    idx16 = nc.dram_tensor(
        "idx16", [128, plan["t_idx"] // 16], mybir.dt.int16, kind="ExternalInput"
    )
    seg = nc.dram_tensor(
        "seg", [128, plan["nblk_tot"] * STRIPE], dt, kind="ExternalInput"
    )
    smat = nc.dram_tensor(
        "smat", [128, plan["tot_pairs"] * SUBW], dt, kind="ExternalInput"
    )
    w1p = nc.dram_tensor("w1p", [HID, HID], dt, kind="ExternalInput")
    w2 = nc.dram_tensor("w2", [HID, HID], dt, kind="ExternalInput")
    w3 = nc.dram_tensor("w3", [HID, HID], dt, kind="ExternalInput")
    wfc = nc.dram_tensor("wfc", [HID, N_CLS], dt, kind="ExternalInput")
    b1 = nc.dram_tensor("b1", [HID, 1], dt, kind="ExternalInput")
    b2 = nc.dram_tensor("b2", [HID, 1], dt, kind="ExternalInput")
    b3 = nc.dram_tensor("b3", [HID, 1], dt, kind="ExternalInput")
    bfc = nc.dram_tensor("bfc", [128, N_CLS], dt, kind="ExternalInput")
    out = nc.dram_tensor("out", [NS, N_CLS], dt, kind="ExternalOutput")

    t2_loc = nc.dram_tensor("t2_loc", [NS, HID], dt)
    t3_loc = nc.dram_tensor("t3_loc", [NS, HID], dt)
    t2_full = nc.dram_tensor("t2_full", [N_NODES, HID], dt, addr_space="Shared")
    t3_full = nc.dram_tensor("t3_full", [N_NODES, HID], dt, addr_space="Shared")

    AF = mybir.ActivationFunctionType
    OP = mybir.AluOpType

    max_pt = max(
        int(-(-(pw0[w + 1] - pw0[w]) // PT_BLKS)) for w in range(N_WIN)
    )

    with TileContext(nc) as tc:
        with (
            tc.tile_pool(name="const", bufs=1) as cpool,
            tc.tile_pool(name="stream", bufs=2) as spool,
            tc.tile_pool(name="msg", bufs=4) as mpool,
            tc.tile_pool(name="pt", bufs=max_pt + 8) as ptpool,
            tc.tile_pool(name="hseg", bufs=2) as hpool,
            tc.tile_pool(name="rows", bufs=3) as rpool,
            tc.tile_pool(name="ppack", bufs=2, space="PSUM") as ppk,
            tc.tile_pool(name="pwin", bufs=2, space="PSUM") as pwn,
            tc.tile_pool(name="ptr", bufs=2, space="PSUM") as ptr,
        ):
            w1s = cpool.tile([HID, HID], dt, name="w1s")
            nc.sync.dma_start(out=w1s[:], in_=w1p[:])
            w2s = cpool.tile([HID, HID], dt, name="w2s")
            nc.sync.dma_start(out=w2s[:], in_=w2[:])
            w3s = cpool.tile([HID, HID], dt, name="w3s")
            nc.sync.dma_start(out=w3s[:], in_=w3[:])
            wfcs = cpool.tile([HID, N_CLS], dt, name="wfcs")
            nc.sync.dma_start(out=wfcs[:], in_=wfc[:])
            b1s = cpool.tile([HID, 1], dt, name="b1s")
            nc.sync.dma_start(out=b1s[:], in_=b1[:])
            b2s = cpool.tile([HID, 1], dt, name="b2s")
            nc.sync.dma_start(out=b2s[:], in_=b2[:])
            b3s = cpool.tile([HID, 1], dt, name="b3s")
            nc.sync.dma_start(out=b3s[:], in_=b3[:])
            bfcs = cpool.tile([128, N_CLS], dt, name="bfcs")
            nc.sync.dma_start(out=bfcs[:], in_=bfc[:])

            def layer(li, table, tnext_loc, bias_s, wnext_s):
                for w in range(N_WIN):
                    icol0 = int(pw0[w]) * (BLK // 16)
                    icols = int(b_max[w].sum()) * (BLK // 16)
                    idx_sl = spool.tile(
                        [128, icols], mybir.dt.int16, name="idx_sl", tag="idx"
                    )
                    nc.sync.dma_start(
                        out=idx_sl[:], in_=idx16[:, icol0 : icol0 + icols]
                    )
                    scol0 = int(pw0[w]) * STRIPE
                    scols = int(b_max[w].sum()) * STRIPE
                    seg_sl = spool.tile([128, scols], dt, name="seg_sl", tag="seg")
                    nc.sync.dma_start(
                        out=seg_sl[:], in_=seg[:, scol0 : scol0 + scols]
                    )
                    np_w = len(pairs[w])
                    sm_sl = spool.tile(
                        [128, np_w * SUBW], dt, name="sm_sl", tag="smat"
                    )
                    pcol0 = int(pair_base[w]) * SUBW
                    nc.sync.dma_start(
                        out=sm_sl[:], in_=smat[:, pcol0 : pcol0 + np_w * SUBW]
                    )

                    pt_sbufs = []
                    pt_psum = None
                    jw = 0
                    for c in range(N_CHUNKS):
                        bmax = int(b_max[w, c])
                        c0 = c * CHUNK
                        c1 = min(c0 + CHUNK, N_NODES)
                        done = 0
                        while done < bmax:
                            nblk = min(CALL_BLKS, bmax - done)
                            nidx = nblk * BLK
                            g0 = int(p0[w, c]) + done
                            coff = (g0 - int(pw0[w])) * (BLK // 16)
                            msg = mpool.tile(
                                [128, CALL_BLKS, HID], dt, name="msg", tag="msg"
                            )
                            nc.gpsimd.dma_gather(
                                out_ap=msg[:, :nblk, :],
                                in_ap=table[c0:c1, :],
                                idxs_ap=idx_sl[:, coff : coff + nidx // 16],
                                num_idxs=nidx,
                                num_idxs_reg=nidx,
                                elem_size=HID,
                            )
                            for jj in range(nblk):
                                if jw % PT_BLKS == 0:
                                    pt_psum = ppk.tile(
                                        [128, HID], dt, name="ptp", tag="ptp"
                                    )
                                prow = (jw % PT_BLKS) * STRIPE
                                nc.tensor.matmul(
                                    out=pt_psum[prow : prow + STRIPE, :],
                                    lhsT=seg_sl[
                                        :, jw * STRIPE : (jw + 1) * STRIPE
                                    ],
                                    rhs=msg[:, jj, :],
                                    start=True,
                                    stop=True,
                                    tile_position=(0, prow),
                                )
                                if jw % PT_BLKS == PT_BLKS - 1:
                                    pts = ptpool.tile(
                                        [128, HID], dt, name="pts", tag="pts"
                                    )
                                    nc.vector.tensor_copy(pts[:], pt_psum[:])
                                    pt_sbufs.append(pts)
                                jw += 1
                            done += nblk

                    win_ps = pwn.tile([HID, WIN], dt, name="win_ps", tag="win")
                    for pi, (s, sw, st, sp) in enumerate(pairs[w]):
                        nc.tensor.matmul(
                            out=win_ps[:, sw * SUBW : (sw + 1) * SUBW],
                            lhsT=pt_sbufs[s][:],
                            rhs=sm_sl[:, pi * SUBW : (pi + 1) * SUBW],
                            start=st,
                            stop=sp,
                        )

                    nw = n_w[w]
                    hT = hpool.tile([HID, WIN], dt, name="hT", tag="hT")
                    if li == 1:
                        agg_s = hpool.tile(
                            [HID, WIN], dt, name="agg_s", tag="agg"
                        )
                        nc.scalar.activation(
                            agg_s[:, :nw], win_ps[:, :nw], AF.Copy
                        )
                        h_ps = ptr.tile([HID, WIN], dt, name="h_ps", tag="hps")
                        nc.tensor.matmul(
                            out=h_ps[:, :nw],
                            lhsT=w1s[:],
                            rhs=agg_s[:, :nw],
                            start=True,
                            stop=True,
                        )
                        src_ps = h_ps
                    else:
                        src_ps = win_ps
                    # leaky_relu(x + b): t0 = x + b; hT = max(t0, 0.01*t0)
                    t0 = hpool.tile([HID, WIN], dt, name="t0", tag="t0")
                    nc.scalar.activation(
                        t0[:, :nw], src_ps[:, :nw], AF.Identity, bias=bias_s[:]
                    )
                    t1 = hpool.tile([HID, WIN], dt, name="t1", tag="t1")
                    nc.vector.tensor_scalar_mul(t1[:, :nw], t0[:, :nw], NEG_SLOPE)
                    nc.vector.tensor_tensor(
                        out=hT[:, :nw],
                        in0=t0[:, :nw],
                        in1=t1[:, :nw],
                        op=OP.max,
                    )

                    t0g = w * WIN
                    for tt in range(0, nw, 128):
                        tlen = min(128, nw - tt)
                        if li < 3:
                            tr = ptr.tile(
                                [128, HID], dt, name="tr", tag="tr"
                            )
                            nc.tensor.matmul(
                                out=tr[:tlen, :],
                                lhsT=hT[:, tt : tt + tlen],
                                rhs=wnext_s[:],
                                start=True,
                                stop=True,
                            )
                            rows = rpool.tile(
                                [128, HID], dt, name="rows", tag="rows"
                            )
                            nc.vector.tensor_copy(rows[:tlen, :], tr[:tlen, :])
                            nc.sync.dma_start(
                                out=tnext_loc[t0g + tt : t0g + tt + tlen, :],
                                in_=rows[:tlen, :],
                            )
                        else:
                            tr = ptr.tile(
                                [128, HID], dt, name="trf", tag="tr"
                            )
                            nc.tensor.matmul(
                                out=tr[:tlen, :N_CLS],
                                lhsT=hT[:, tt : tt + tlen],
                                rhs=wfcs[:],
                                start=True,
                                stop=True,
                            )
                            rows = rpool.tile(
                                [128, N_CLS], dt, name="rowsf", tag="rowsf"
                            )
                            nc.vector.tensor_tensor(
                                out=rows[:tlen, :],
                                in0=tr[:tlen, :N_CLS],
                                in1=bfcs[:tlen, :],
                                op=OP.add,
                            )
                            nc.sync.dma_start(
                                out=out[t0g + tt : t0g + tt + tlen, :],
                                in_=rows[:tlen, :],
                            )

            for _rep in range(reps):
                layer(1, x_pad, t2_loc, b1s, w2s)
                allgather(t2_loc, t2_full)
                layer(2, t2_full, t3_loc, b2s, w3s)
                allgather(t3_loc, t3_full)
                layer(3, t3_full, None, b3s, None)

    nc.finalize()
    return nc


# ---------------------------------------------------------------------------
# PJRT SPMD runner (build once, run many)
# ---------------------------------------------------------------------------


class _Runner:
    def __init__(self, nc, n_cores):
        import jax
        from jax.sharding import Mesh, PartitionSpec
        from jax.experimental.shard_map import shard_map
        from concourse import mybir
        from concourse.bass2jax import (
            _bass_exec_p,
            install_neuronx_cc_hook,
            partition_id_tensor,
        )

        install_neuronx_cc_hook()
        self.jax = jax
        self.n_cores = n_cores
        partition_name = (
            nc.partition_id_tensor.name if nc.partition_id_tensor else None
        )
        in_names, out_names, out_avals, zero_outs = [], [], [], []
        for alloc in nc.m.functions[0].allocations:
            if not isinstance(alloc, mybir.MemoryLocationSet):
                continue
            name = alloc.memorylocations[0].name
            if alloc.kind == "ExternalInput":
                if name != partition_name:
                    in_names.append(name)
            elif alloc.kind == "ExternalOutput":
                shape = tuple(alloc.tensor_shape)
                dtype = mybir.dt.np(alloc.dtype)
                out_names.append(name)
                out_avals.append(jax.core.ShapedArray(shape, dtype))
                zero_outs.append(np.zeros(shape, dtype))
        n_params = len(in_names)
        in_names = in_names + out_names
        if partition_name is not None:
            in_names.append(partition_name)
        self.in_names, self.n_params = in_names, n_params
        self.out_names, self.out_avals = out_names, out_avals
        self.zero_outs = zero_outs

        def _body(*args):
            operands = list(args)
            if partition_name is not None:
                operands.append(partition_id_tensor())
            return tuple(
                _bass_exec_p.bind(
                    *operands,
                    out_avals=tuple(out_avals),
                    in_names=tuple(in_names),
                    out_names=tuple(out_names),
                    lowering_input_output_aliases=(),
                    sim_require_finite=True,
                    sim_require_nnan=True,
                    nc=nc,
                )
            )

        devices = jax.devices()[:n_cores]
        self.mesh = Mesh(np.asarray(devices), ("core",))
        self.devices = devices
        self.PartitionSpec = PartitionSpec
        n_outs = len(out_avals)
        self.sharded = jax.jit(
            shard_map(
                _body,
                mesh=self.mesh,
                in_specs=(PartitionSpec("core"),) * (n_params + n_outs),
                out_specs=(PartitionSpec("core"),) * n_outs,
                check_rep=False,
            ),
            donate_argnums=tuple(range(n_params, n_params + n_outs)),
            keep_unused=True,
        )

    def prepare(self, in_maps):
        from jax.sharding import NamedSharding

        jax = self.jax
        n = self.n_cores
        sh = NamedSharding(self.mesh, self.PartitionSpec("core"))
        put = []
        for name in self.in_names[: self.n_params]:
            x = np.concatenate(
                [np.asarray(m[name]) for m in in_maps], axis=0
            )
            shards = np.split(x, n, axis=0)
            bufs = [
                jax.device_put(s, d)
                for s, d in zip(shards, self.devices, strict=True)
            ]
            put.append(
                jax.make_array_from_single_device_arrays(x.shape, sh, bufs)
            )
        jax.block_until_ready(put)
        return put

    def run(self, concat_in):
        n = self.n_cores
        zeros = [
            np.zeros((n * z.shape[0], *z.shape[1:]), z.dtype)
            for z in self.zero_outs
        ]
        outs = self.sharded(*concat_in, *zeros)
        self.jax.block_until_ready(outs)
        return outs

    def results(self, outs):
        n = self.n_cores
        return [
            {
                name: np.asarray(outs[i]).reshape(n, *self.out_avals[i].shape)[
                    c
                ]
                for i, name in enumerate(self.out_names)
            }
            for c in range(n)
        ]


# ---------------------------------------------------------------------------
# Entry point
# ---------------------------------------------------------------------------


def make_in_maps(plan, x, W1, b1, W2, b2, W3, b3, Wfc, bfc):
    x_pad = np.zeros((N_NODES, HID), np.float32)
    x_pad[:, :IN_F] = np.asarray(x, np.float32)
    w1p = np.zeros((HID, HID), np.float32)
    w1p[:IN_F, :] = np.asarray(W1, np.float32)
    base = dict(
        x_pad=x_pad,
        w1p=w1p,
        w2=np.asarray(W2, np.float32),
        w3=np.asarray(W3, np.float32),
        wfc=np.asarray(Wfc, np.float32),
        b1=np.asarray(b1, np.float32).reshape(HID, 1),
        b2=np.asarray(b2, np.float32).reshape(HID, 1),
        b3=np.asarray(b3, np.float32).reshape(HID, 1),
        bfc=np.tile(np.asarray(bfc, np.float32).reshape(1, N_CLS), (128, 1)),
    )
    return [
        dict(
            base,
            idx16=plan["idx"][r],
            seg=plan["seg"][r],
            smat=plan["smat"][r],
        )
        for r in range(N_CORES)
    ]


_CACHE = {}


def get_runner(plan, reps=1):
    key = ("nc", reps)
    if key not in _CACHE:
        nc = build_nc(plan, reps=reps)
        _CACHE[key] = _Runner(nc, N_CORES)
    return _CACHE[key]


def kernel(x, edge_index, W1, b1, W2, b2, W3, b3, Wfc, bfc):
    plan = build_plan(edge_index)
    runner = get_runner(plan, reps=1)
    in_maps = make_in_maps(plan, x, W1, b1, W2, b2, W3, b3, Wfc, bfc)
    ci = runner.prepare(in_maps)
    res = runner.results(runner.run(ci))
    return np.concatenate([res[r]["out"] for r in range(N_CORES)], axis=0)



# revision 2
# speedup vs baseline: 1.2883x; 1.2883x over previous
"""3-layer GCN (PyG GCNConv x3 + FC) on 8 Trainium2 NeuronCores — v2.

Key ideas vs v1:
  - Separable GCN norm: dinv[s]*dinv[d] = (row-prescale of the table by
    dinv[s]) x (column-postscale of the aggregate by dinv[d]). The per-edge
    norm streams (seg) and the 0/1 merge streams (smat) disappear from DRAM.
  - One-hot stage-1/stage-2 matmul operands are GENERATED ON-CHIP by DVE
    is_equal against constant ramp tiles, from 2-byte-per-edge column-index
    streams.
  - Table rows are fp16 padded to 128 elems (256B — the dma_gather minimum);
    fp16 matmuls run 4x faster than fp32 on the PE.
  - Transform + bias + leaky-relu happen feature-major ([64, 512] tiles):
    bias and Lrelu fuse into one Activation op; dinv column scales are
    tensor_tensor multiplies against an SBUF-resident dinv table.
  - Final FC output stays feature-major [10, NS]; the host transposes and
    adds bfc.

Sharding: core r owns destination nodes [r*12500, (r+1)*12500) and their
incoming edges (plus self loops). Weights replicated. AllGather (fp16
tables) between layers. All 8 cores run ONE shared instruction stream.
"""

import sys

sys.path.insert(0, "/opt/trn_rl_repo")

import numpy as np

N_NODES = 100000
N_EDGES = 3200000
IN_F, HID, N_CLS = 10, 64, 10
NEG_SLOPE = 0.01
N_CORES = 8
NS = N_NODES // N_CORES  # 12500 dst nodes per core
CHUNK = 32768  # table rows per gather chunk (int16 index range)
N_CHUNKS = (N_NODES + CHUNK - 1) // CHUNK  # 4
WIN = 512  # dst nodes per window
N_WIN = (NS + WIN - 1) // WIN  # 25
BLK = 128  # edges per block (PE contraction dim)
STRIPE = 32  # dst stripe positions per block (tile_position granularity)
PT_BLKS = 4  # blocks per packed psum tile (4*32 = 128 rows)
CALL_BLKS = 8  # blocks per dma_gather call (1024 idx: HW SWDGE ring limit)
SUBW = 128  # dst per merge subwindow
ROWP = 128  # padded table row elems (fp16 -> 256B, dma_gather minimum)
IN_W1 = 16  # layer-1 feature width (10 padded to 16)
DMA_SCRATCH = 16384  # SWDGE ring: 1024 descriptors (HW-fixed)


# ---------------------------------------------------------------------------
# Host-side packing
# ---------------------------------------------------------------------------


def build_plan(edge_index):
    src = np.asarray(edge_index[0], dtype=np.int64)
    dst = np.asarray(edge_index[1], dtype=np.int64)
    deg = np.bincount(dst, minlength=N_NODES).astype(np.float64) + 1.0
    dinv = (1.0 / np.sqrt(deg)).astype(np.float32)
    loop = np.arange(N_NODES, dtype=np.int64)
    s_all = np.concatenate([src, loop])
    d_all = np.concatenate([dst, loop])

    cores = []
    for r in range(N_CORES):
        sel = (d_all >= r * NS) & (d_all < (r + 1) * NS)
        es, ed = s_all[sel], d_all[sel] - r * NS
        w_id = ed // WIN
        c_id = es // CHUNK
        order = np.lexsort((ed, c_id, w_id))
        es, ed, w_id, c_id = es[order], ed[order], w_id[order], c_id[order]
        key = w_id * N_CHUNKS + c_id
        run_starts = np.searchsorted(key, np.arange(N_WIN * N_CHUNKS), "left")
        run_ends = np.searchsorted(key, np.arange(N_WIN * N_CHUNKS), "right")

        n_e = len(es)
        newd = np.empty(n_e, dtype=bool)
        newd[0] = True
        newd[1:] = (ed[1:] != ed[:-1]) | (key[1:] != key[:-1])
        rank = np.cumsum(newd) - 1
        first_occ = np.flatnonzero(newd)
        blocks = {}
        for w in range(N_WIN):
            for c in range(N_CHUNKS):
                a, b = run_starts[w * N_CHUNKS + c], run_ends[w * N_CHUNKS + c]
                lst = []
                p = a
                while p < b:
                    r0 = rank[p]
                    lim_rank = r0 + STRIPE
                    lim = first_occ[lim_rank] if lim_rank <= rank[b - 1] else b
                    q = min(p + BLK, lim, b)
                    lst.append((p, q, r0))
                    p = q
                blocks[(w, c)] = lst
        cores.append(dict(es=es, ed=ed, rank=rank, blocks=blocks))

    # ---- global uniform structure -----------------------------------------
    b_max = np.zeros((N_WIN, N_CHUNKS), dtype=np.int64)
    for r in range(N_CORES):
        for w in range(N_WIN):
            for c in range(N_CHUNKS):
                b_max[w, c] = max(b_max[w, c], len(cores[r]["blocks"][(w, c)]))
    for w in range(N_WIN):
        extra = (-b_max[w].sum()) % PT_BLKS
        b_max[w, N_CHUNKS - 1] += extra

    p0 = np.zeros((N_WIN, N_CHUNKS), dtype=np.int64)
    acc = 0
    pw0 = np.zeros(N_WIN + 1, dtype=np.int64)
    for w in range(N_WIN):
        pw0[w] = acc
        for c in range(N_CHUNKS):
            p0[w, c] = acc
            acc += b_max[w, c]
    pw0[N_WIN] = acc
    nblk_tot = int(acc)
    nslab = [(int(pw0[w + 1] - pw0[w]) + PT_BLKS - 1) // PT_BLKS for w in range(N_WIN)]
    n_w = [min(WIN, NS - w * WIN) for w in range(N_WIN)]
    nsw = [(n_w[w] + SUBW - 1) // SUBW for w in range(N_WIN)]

    # ---- per-core stream arrays + pair discovery --------------------------
    t_idx = nblk_tot * BLK
    idx_streams = np.zeros((N_CORES, t_idx), dtype=np.int16)
    valid_streams = np.zeros((N_CORES, t_idx), dtype=bool)
    colidx = np.full((N_CORES, 128, nblk_tot), -1, dtype=np.int16)
    # packed col (slab row) -> window-local dst, per core (host only)
    pc2dst = np.full((N_CORES, nblk_tot * STRIPE), -1, dtype=np.int64)
    pair_sets = [set() for _ in range(N_WIN)]

    for r in range(N_CORES):
        co = cores[r]
        es, ed, rank = co["es"], co["ed"], co["rank"]
        for w in range(N_WIN):
            for c in range(N_CHUNKS):
                for j, (a, b, r0) in enumerate(co["blocks"][(w, c)]):
                    g = int(p0[w, c]) + j
                    sl = slice(a, b)
                    slot = np.arange(b - a)
                    col = rank[sl] - r0
                    idx_streams[r][g * BLK + slot] = (es[sl] - c * CHUNK).astype(
                        np.int16
                    )
                    valid_streams[r][g * BLK + slot] = True
                    colidx[r][slot, g] = col.astype(np.int16)
                    pc2dst[r][g * STRIPE + col] = ed[sl]  # window-local later
        # pairs: slab s of window w merges into subwindow sw
        for w in range(N_WIN):
            g0, g1 = int(pw0[w]), int(pw0[w + 1])
            pcs = np.arange(g0 * STRIPE, g1 * STRIPE)
            d = pc2dst[r][pcs]
            valid = d >= 0
            if valid.any():
                dl = d[valid] - w * WIN
                s_local = (pcs[valid] - g0 * STRIPE) // 128
                for s, sw in set(zip(s_local, dl // SUBW)):
                    pair_sets[w].add((int(s), int(sw)))

    pairs = []
    pair_base = np.zeros(N_WIN + 1, dtype=np.int64)
    tot_pairs = 0
    for w in range(N_WIN):
        ordered = sorted(pair_sets[w], key=lambda t: (t[1], t[0]))
        lst = []
        for i, (s, sw) in enumerate(ordered):
            start = i == 0 or ordered[i - 1][1] != sw
            stop = i == len(ordered) - 1 or ordered[i + 1][1] != sw
            lst.append((s, sw, start, stop))
        pairs.append(lst)
        pair_base[w] = tot_pairs
        tot_pairs += len(lst)
    pair_base[N_WIN] = tot_pairs

    # stage-2 column-index stream: pcsub[p, gp] = window-local dst - sw*SUBW
    pcsub = np.full((N_CORES, 128, tot_pairs), -1, dtype=np.float32)
    for r in range(N_CORES):
        for w in range(N_WIN):
            g0 = int(pw0[w])
            for pi, (s, sw, _a, _b) in enumerate(pairs[w]):
                gp = int(pair_base[w]) + pi
                rows = np.arange(128)
                pcs = (g0 * STRIPE) + s * 128 + rows
                d = pc2dst[r][pcs]
                m = (d >= 0) & ((d - w * WIN) // SUBW == sw)
                pcsub[r][rows[m], gp] = ((d[m] - w * WIN) - sw * SUBW).astype(
                    np.float32
                )

    # ---- -1 trailing padding + per-call valid counts ----------------------
    # device call order: for w: for c: calls of <= CALL_BLKS blocks
    call_meta = []  # (w, c, slot0, nidx)
    for w in range(N_WIN):
        for c in range(N_CHUNKS):
            bmax = int(b_max[w, c])
            done = 0
            while done < bmax:
                nblk = min(CALL_BLKS, bmax - done)
                call_meta.append(
                    (w, c, (int(p0[w, c]) + done) * BLK, nblk * BLK)
                )
                done += nblk
    n_calls = len(call_meta)
    ncfg = np.zeros((N_CORES, 1, n_calls), dtype=np.int32)
    for r in range(N_CORES):
        iflat = idx_streams[r]
        vflat = valid_streams[r]
        for w in range(N_WIN):
            for c in range(N_CHUNKS):
                gs0 = int(p0[w, c]) * BLK
                gs1 = gs0 + int(b_max[w, c]) * BLK
                vm = vflat[gs0:gs1]
                lt = int(np.flatnonzero(vm)[-1]) if vm.any() else -1
                iflat[gs0 + lt + 1 : gs1] = -1
        for ci, (w, c, s0, nidx) in enumerate(call_meta):
            cnt = int((iflat[s0 : s0 + nidx] >= 0).sum())
            if cnt == 0:
                iflat[s0] = 0
                cnt = 1
            else:
                # suffix-only check: all valid slots precede all -1 slots
                nn = iflat[s0 : s0 + nidx] >= 0
                assert nn[:cnt].all(), (r, w, c, "interior -1 in call")
            ncfg[r, 0, ci] = cnt

    idx16 = np.ascontiguousarray(
        np.tile(idx_streams.reshape(N_CORES, -1, 16).transpose(0, 2, 1), (1, 8, 1))
    )  # [cores, 128, t_idx//16]

    # constants
    ramp32 = np.tile(np.arange(STRIPE, dtype=np.int16)[None, :], (128, 1))
    ramp128 = np.tile(np.arange(SUBW, dtype=np.float32)[None, :], (128, 1))
    dinvrep = np.zeros((N_CORES, 64, NS), dtype=np.float32)
    for r in range(N_CORES):
        dinvrep[r] = np.tile(dinv[r * NS : (r + 1) * NS][None, :], (64, 1))

    return dict(
        b_max=b_max,
        p0=p0,
        pw0=pw0,
        n_w=n_w,
        nslab=nslab,
        nsw=nsw,
        pairs=pairs,
        pair_base=pair_base,
        nblk_tot=nblk_tot,
        t_idx=t_idx,
        tot_pairs=int(tot_pairs),
        idx=idx16,
        ncfg=ncfg,
        n_calls=n_calls,
        colidx=colidx,
        pcsub=pcsub,
        ramp32=ramp32,
        ramp128=ramp128,
        dinvrep=dinvrep,
        dinv=dinv,
    )


# ---------------------------------------------------------------------------
# Kernel builder
# ---------------------------------------------------------------------------


def build_nc(plan, reps=1, sim_mode=False):
    import concourse.bacc as bacc
    from concourse import mybir
    from concourse.tile import TileContext

    f32 = mybir.dt.float32
    f16 = mybir.dt.float16
    i16 = mybir.dt.int16
    b_max = plan["b_max"]
    p0 = plan["p0"]
    pw0 = plan["pw0"]
    pairs = plan["pairs"]
    pair_base = plan["pair_base"]
    n_w = plan["n_w"]
    nslab = plan["nslab"]

    nc = bacc.Bacc(
        "TRN2",
        num_devices=1 if sim_mode else N_CORES,
        dynamic_dma_scratch_size=DMA_SCRATCH,
    )

    def allgather(t_loc, t_full):
        if sim_mode:
            nc.sync.dma_start(out=t_full[0:NS, :], in_=t_loc[:])
        else:
            nc.gpsimd.collective_compute(
                "AllGather",
                mybir.AluOpType.bypass,
                replica_groups=[list(range(N_CORES))],
                ins=[t_loc[:]],
                outs=[t_full[:]],
            )

    xh = nc.dram_tensor("xh", [N_NODES, ROWP], f16, kind="ExternalInput")
    idx16 = nc.dram_tensor(
        "idx16", [128, plan["t_idx"] // 16], i16, kind="ExternalInput"
    )
    colidx = nc.dram_tensor(
        "colidx", [128, plan["nblk_tot"]], i16, kind="ExternalInput"
    )
    ncfg_d = nc.dram_tensor(
        "ncfg", [1, plan["n_calls"]], mybir.dt.int32, kind="ExternalInput"
    )
    pcsub = nc.dram_tensor(
        "pcsub", [128, max(plan["tot_pairs"], 1)], f32, kind="ExternalInput"
    )
    ramp32_d = nc.dram_tensor("ramp32", [128, STRIPE], i16, kind="ExternalInput")
    ramp128_d = nc.dram_tensor("ramp128", [128, SUBW], f32, kind="ExternalInput")
    ident_d = nc.dram_tensor("ident64", [HID, HID], f16, kind="ExternalInput")
    dinvrep_d = nc.dram_tensor("dinvrep", [64, NS], f32, kind="ExternalInput")
    w1p = nc.dram_tensor("w1p", [HID, HID], f32, kind="ExternalInput")
    w2 = nc.dram_tensor("w2", [HID, HID], f32, kind="ExternalInput")
    w3 = nc.dram_tensor("w3", [HID, HID], f32, kind="ExternalInput")
    wfc = nc.dram_tensor("wfc", [HID, N_CLS], f32, kind="ExternalInput")
    b1 = nc.dram_tensor("b1", [HID, 1], f32, kind="ExternalInput")
    b2 = nc.dram_tensor("b2", [HID, 1], f32, kind="ExternalInput")
    b3 = nc.dram_tensor("b3", [HID, 1], f32, kind="ExternalInput")
    outT = nc.dram_tensor("outT", [N_CLS, NS], f32, kind="ExternalOutput")

    t2_loc = nc.dram_tensor("t2_loc", [NS, ROWP], f16)
    t3_loc = nc.dram_tensor("t3_loc", [NS, ROWP], f16)
    if sim_mode:
        # numerical single-core check: layer-2/3 tables come from the host
        t2_full = nc.dram_tensor(
            "t2_full", [N_NODES, ROWP], f16, kind="ExternalInput"
        )
        t3_full = nc.dram_tensor(
            "t3_full", [N_NODES, ROWP], f16, kind="ExternalInput"
        )
        t2_out = nc.dram_tensor("t2_out", [NS, ROWP], f16, kind="ExternalOutput")
        t3_out = nc.dram_tensor("t3_out", [NS, ROWP], f16, kind="ExternalOutput")
    else:
        t2_full = nc.dram_tensor(
            "t2_full", [N_NODES, ROWP], f16, addr_space="Shared"
        )
        t3_full = nc.dram_tensor(
            "t3_full", [N_NODES, ROWP], f16, addr_space="Shared"
        )

    AF = mybir.ActivationFunctionType
    OP = mybir.AluOpType

    max_pt = max(nslab) + 6

    with TileContext(nc) as tc:
        with (
            tc.tile_pool(name="const", bufs=1) as cpool,
            tc.tile_pool(name="stream", bufs=2) as spool,
            tc.tile_pool(name="msg", bufs=4) as mpool,
            tc.tile_pool(name="seg", bufs=4) as gpool,
            tc.tile_pool(name="pt", bufs=max_pt) as ptpool,
            tc.tile_pool(name="ysc", bufs=2) as ypool,
            tc.tile_pool(name="hrow", bufs=3) as rpool,
            tc.tile_pool(name="ppack", bufs=2, space="PSUM") as ppk,
            tc.tile_pool(name="pwin", bufs=2, space="PSUM") as pwn,
            tc.tile_pool(name="ptr", bufs=2, space="PSUM") as ptr,
            tc.tile_pool(name="ptt", bufs=2, space="PSUM") as ptt,
        ):
            ramp32_s = cpool.tile([128, STRIPE], i16, name="ramp32_s")
            nc.sync.dma_start(out=ramp32_s[:], in_=ramp32_d[:])
            ramp128_s = cpool.tile([128, SUBW], f32, name="ramp128_s")
            nc.sync.dma_start(out=ramp128_s[:], in_=ramp128_d[:])
            ident_s = cpool.tile([HID, HID], f16, name="ident_s")
            nc.sync.dma_start(out=ident_s[:], in_=ident_d[:])
            dinvrep_s = cpool.tile([64, NS], f32, name="dinvrep_s")
            nc.sync.dma_start(out=dinvrep_s[:], in_=dinvrep_d[:])
            w1s = cpool.tile([HID, HID], f32, name="w1s")
            nc.sync.dma_start(out=w1s[:], in_=w1p[:])
            w2s = cpool.tile([HID, HID], f32, name="w2s")
            nc.sync.dma_start(out=w2s[:], in_=w2[:])
            w3s = cpool.tile([HID, HID], f32, name="w3s")
            nc.sync.dma_start(out=w3s[:], in_=w3[:])
            wfcs = cpool.tile([HID, N_CLS], f32, name="wfcs")
            nc.sync.dma_start(out=wfcs[:], in_=wfc[:])
            b1s = cpool.tile([HID, 1], f32, name="b1s")
            nc.sync.dma_start(out=b1s[:], in_=b1[:])
            b2s = cpool.tile([HID, 1], f32, name="b2s")
            nc.sync.dma_start(out=b2s[:], in_=b2[:])
            b3s = cpool.tile([HID, 1], f32, name="b3s")
            nc.sync.dma_start(out=b3s[:], in_=b3[:])
            ncfg_s = cpool.tile([1, plan["n_calls"]], mybir.dt.int32, name="ncfg_s")
            nc.sync.dma_start(out=ncfg_s[:], in_=ncfg_d[:])
            msgP = []
            for mi in range(4):
                mt = cpool.tile(
                    [128, CALL_BLKS, ROWP], f16, name=f"msgP{mi}"
                )
                nc.vector.memset(mt[:], 0.0)
                msgP.append(mt)
            call_ctr = [0]
            nv_regs = [
                nc.gpsimd.alloc_register(f"nvreg{i}") for i in range(8)
            ]

            def layer(li, table, tnext_loc, bias_s, w_s):
                in_w = HID
                """aggregate table -> y=dinv*agg -> z=W^T y -> h=Lrelu(z+b)
                -> (layers 1-2) table rows ĥ=dinv*h, transposed out; or
                (layer 3) outT = Wfc^T h."""
                for w in range(N_WIN):
                    nw = n_w[w]
                    wblocks = int(b_max[w].sum())
                    icol0 = int(pw0[w]) * (BLK // 16)
                    icols = wblocks * (BLK // 16)
                    idx_sl = spool.tile([128, icols], i16, name="idx_sl", tag="idx")
                    nc.sync.dma_start(
                        out=idx_sl[:], in_=idx16[:, icol0 : icol0 + icols]
                    )
                    col_sl = spool.tile(
                        [128, wblocks], i16, name="col_sl", tag="col"
                    )
                    nc.sync.dma_start(
                        out=col_sl[:],
                        in_=colidx[:, int(pw0[w]) : int(pw0[w]) + wblocks],
                    )
                    np_w = len(pairs[w])
                    pcs_sl = spool.tile(
                        [128, max(np_w, 1)], f32, name="pcs_sl", tag="pcs"
                    )
                    pb = int(pair_base[w])
                    nc.sync.dma_start(
                        out=pcs_sl[:, :np_w], in_=pcsub[:, pb : pb + np_w]
                    )

                    pt_sbufs = []
                    pt_psum = None
                    jw = 0
                    for c in range(N_CHUNKS):
                        bmax = int(b_max[w, c])
                        c0 = c * CHUNK
                        c1 = min(c0 + CHUNK, N_NODES)
                        done = 0
                        while done < bmax:
                            nblk = min(CALL_BLKS, bmax - done)
                            nidx = nblk * BLK
                            g0 = int(p0[w, c]) + done
                            coff = (g0 - int(pw0[w])) * (BLK // 16)
                            ci = call_ctr[0] % plan["n_calls"]
                            call_ctr[0] += 1
                            msg = msgP[ci % 4]
                            nv = nv_regs[ci % 8]
                            nc.gpsimd.reg_load(nv, ncfg_s[0:1, ci : ci + 1])
                            nc.gpsimd.dma_gather(
                                out_ap=msg[:, :nblk, :],
                                in_ap=table[c0:c1, :],
                                idxs_ap=idx_sl[:, coff : coff + nidx // 16],
                                num_idxs=nidx,
                                num_idxs_reg=nv,
                                elem_size=ROWP,
                            )
                            for jj in range(nblk):
                                if jw % PT_BLKS == 0:
                                    pt_psum = ppk.tile(
                                        [128, in_w], f32, name="ptp", tag="ptp"
                                    )
                                    nseg = min(PT_BLKS, wblocks - jw)
                                    seg = gpool.tile(
                                        [128, PT_BLKS, STRIPE],
                                        f16,
                                        name="seg",
                                        tag="seg",
                                    )
                                    jws = jw  # window-block index of seg[:, 0]
                                    nc.vector.tensor_tensor(
                                        out=seg[:, :nseg, :],
                                        in0=ramp32_s[:, :]
                                        .unsqueeze(1)
                                        .broadcast_to([128, nseg, STRIPE]),
                                        in1=col_sl[:, jws : jws + nseg]
                                        .unsqueeze(2)
                                        .broadcast_to([128, nseg, STRIPE]),
                                        op=OP.is_equal,
                                    )
                                prow = (jw % PT_BLKS) * STRIPE
                                nc.tensor.matmul(
                                    out=pt_psum[prow : prow + STRIPE, :],
                                    lhsT=seg[:, jw - jws, :],
                                    rhs=msg[:, jj, :in_w],
                                    start=True,
                                    stop=True,
                                    tile_position=(0, prow),
                                )
                                if jw % PT_BLKS == PT_BLKS - 1:
                                    pts = ptpool.tile(
                                        [128, in_w], f16, name="pts", tag="pts"
                                    )
                                    nc.scalar.activation(
                                        pts[:], pt_psum[:], AF.Copy
                                    )
                                    pt_sbufs.append(pts)
                                jw += 1
                            done += nblk

                    # stage 2: merge packed tiles into the window (feature-major)
                    win_ps = pwn.tile([in_w, WIN], f32, name="win_ps", tag="win")
                    for pi, (s, sw, st, sp) in enumerate(pairs[w]):
                        sub_nw = min(SUBW, nw - sw * SUBW)
                        smat = gpool.tile(
                            [128, SUBW], f16, name="smat", tag="smat"
                        )
                        nc.vector.tensor_scalar(
                            out=smat[:, :sub_nw],
                            in0=ramp128_s[:, :sub_nw],
                            scalar1=pcs_sl[:, pi : pi + 1],
                            scalar2=None,
                            op0=OP.is_equal,
                        )
                        nc.tensor.matmul(
                            out=win_ps[:, sw * SUBW : sw * SUBW + sub_nw],
                            lhsT=pt_sbufs[s][:],
                            rhs=smat[:, :sub_nw],
                            start=st,
                            stop=sp,
                        )

                    # y = dinv_d (.) agg   [in_w, nw] f32
                    w0g = w * WIN
                    y = ypool.tile([in_w, WIN], f32, name="y", tag="y")
                    nc.vector.tensor_tensor(
                        out=y[:, :nw],
                        in0=win_ps[:, :nw],
                        in1=dinvrep_s[:in_w, w0g : w0g + nw],
                        op=OP.mult,
                    )
                    # z = W^T y  [HID, nw] f32 (psum)
                    z_ps = ptr.tile([HID, WIN], f32, name="z_ps", tag="z")
                    nc.tensor.matmul(
                        out=z_ps[:, :nw],
                        lhsT=w_s[:],
                        rhs=y[:, :nw],
                        start=True,
                        stop=True,
                    )
                    if li < 3:
                        # h = leaky(z + b); ĥ = dinv_d (.) h -> f16 rows
                        t0 = ypool.tile([HID, WIN], f32, name="t0", tag="t0")
                        nc.scalar.activation(
                            t0[:, :nw], z_ps[:, :nw], AF.Identity, bias=bias_s[:]
                        )
                        t1 = ypool.tile([HID, WIN], f32, name="t1", tag="t1")
                        nc.vector.tensor_scalar_mul(
                            t1[:, :nw], t0[:, :nw], NEG_SLOPE
                        )
                        h = ypool.tile([HID, WIN], f32, name="h", tag="h")
                        nc.vector.tensor_tensor(
                            out=h[:, :nw],
                            in0=t0[:, :nw],
                            in1=t1[:, :nw],
                            op=OP.max,
                        )
                        hh = ypool.tile([HID, WIN], f16, name="hh", tag="hh")
                        nc.vector.tensor_tensor(
                            out=hh[:, :nw],
                            in0=h[:, :nw],
                            in1=dinvrep_s[:, w0g : w0g + nw],
                            op=OP.mult,
                        )
                        # transpose [HID, 128] slices -> [128, HID] rows
                        for tt in range(0, nw, 128):
                            tlen = min(128, nw - tt)
                            tr_ps = ptt.tile([128, HID], f16, name="tr", tag="tr")
                            nc.tensor.transpose(
                                out=tr_ps[:tlen, :],
                                in_=hh[:, tt : tt + tlen],
                                identity=ident_s[:],
                            )
                            rows = rpool.tile(
                                [128, HID], f16, name="rows", tag="rows"
                            )
                            nc.scalar.activation(
                                rows[:tlen, :], tr_ps[:tlen, :], AF.Copy
                            )
                            nc.sync.dma_start(
                                out=tnext_loc[
                                    w0g + tt : w0g + tt + tlen, :HID
                                ],
                                in_=rows[:tlen, :],
                            )
                    else:
                        # h3 = leaky(z+b); outT = Wfc^T h3  [N_CLS, nw]
                        t0 = ypool.tile([HID, WIN], f32, name="t0", tag="t0")
                        nc.scalar.activation(
                            t0[:, :nw], z_ps[:, :nw], AF.Identity, bias=bias_s[:]
                        )
                        t1 = ypool.tile([HID, WIN], f32, name="t1", tag="t1")
                        nc.vector.tensor_scalar_mul(
                            t1[:, :nw], t0[:, :nw], NEG_SLOPE
                        )
                        h = ypool.tile([HID, WIN], f32, name="h3", tag="h")
                        nc.vector.tensor_tensor(
                            out=h[:, :nw],
                            in0=t0[:, :nw],
                            in1=t1[:, :nw],
                            op=OP.max,
                        )
                        o_ps = ptr.tile([N_CLS, WIN], f32, name="o_ps", tag="z")
                        nc.tensor.matmul(
                            out=o_ps[:, :nw],
                            lhsT=wfcs[:],
                            rhs=h[:, :nw],
                            start=True,
                            stop=True,
                        )
                        orow = rpool.tile(
                            [N_CLS, WIN], f32, name="orow", tag="orow"
                        )
                        nc.vector.tensor_copy(orow[:, :nw], o_ps[:, :nw])
                        nc.sync.dma_start(
                            out=outT[:, w0g : w0g + nw], in_=orow[:, :nw]
                        )

            for _rep in range(reps):
                layer(1, xh, t2_loc, b1s, w1s)
                if sim_mode:
                    nc.sync.dma_start(out=t2_out[:], in_=t2_loc[:])
                allgather(t2_loc, t2_full)
                layer(2, t2_full, t3_loc, b2s, w2s)
                if sim_mode:
                    nc.sync.dma_start(out=t3_out[:], in_=t3_loc[:])
                allgather(t3_loc, t3_full)
                layer(3, t3_full, None, b3s, w3s)

    nc.finalize()
    return nc


# ---------------------------------------------------------------------------
# PJRT SPMD runner (build once, run many)
# ---------------------------------------------------------------------------


class _Runner:
    def __init__(self, nc, n_cores):
        import jax
        from jax.sharding import Mesh, PartitionSpec
        from jax.experimental.shard_map import shard_map
        from concourse import mybir
        from concourse.bass2jax import (
            _bass_exec_p,
            install_neuronx_cc_hook,
            partition_id_tensor,
        )

        install_neuronx_cc_hook()
        self.jax = jax
        self.n_cores = n_cores
        partition_name = (
            nc.partition_id_tensor.name if nc.partition_id_tensor else None
        )
        in_names, out_names, out_avals, zero_outs = [], [], [], []
        for alloc in nc.m.functions[0].allocations:
            if not isinstance(alloc, mybir.MemoryLocationSet):
                continue
            name = alloc.memorylocations[0].name
            if alloc.kind == "ExternalInput":
                if name != partition_name:
                    in_names.append(name)
            elif alloc.kind == "ExternalOutput":
                shape = tuple(alloc.tensor_shape)
                dtype = mybir.dt.np(alloc.dtype)
                out_names.append(name)
                out_avals.append(jax.core.ShapedArray(shape, dtype))
                zero_outs.append(np.zeros(shape, dtype))
        n_params = len(in_names)
        in_names = in_names + out_names
        if partition_name is not None:
            in_names.append(partition_name)
        self.in_names, self.n_params = in_names, n_params
        self.out_names, self.out_avals = out_names, out_avals
        self.zero_outs = zero_outs

        def _body(*args):
            operands = list(args)
            if partition_name is not None:
                operands.append(partition_id_tensor())
            return tuple(
                _bass_exec_p.bind(
                    *operands,
                    out_avals=tuple(out_avals),
                    in_names=tuple(in_names),
                    out_names=tuple(out_names),
                    lowering_input_output_aliases=(),
                    sim_require_finite=True,
                    sim_require_nnan=True,
                    nc=nc,
                )
            )

        devices = jax.devices()[:n_cores]
        self.mesh = Mesh(np.asarray(devices), ("core",))
        self.devices = devices
        self.PartitionSpec = PartitionSpec
        n_outs = len(out_avals)
        self.sharded = jax.jit(
            shard_map(
                _body,
                mesh=self.mesh,
                in_specs=(PartitionSpec("core"),) * (n_params + n_outs),
                out_specs=(PartitionSpec("core"),) * n_outs,
                check_rep=False,
            ),
            donate_argnums=tuple(range(n_params, n_params + n_outs)),
            keep_unused=True,
        )

    def prepare(self, in_maps):
        from jax.sharding import NamedSharding

        jax = self.jax
        n = self.n_cores
        sh = NamedSharding(self.mesh, self.PartitionSpec("core"))
        put = []
        for name in self.in_names[: self.n_params]:
            x = np.concatenate([np.asarray(m[name]) for m in in_maps], axis=0)
            shards = np.split(x, n, axis=0)
            bufs = [
                jax.device_put(s, d)
                for s, d in zip(shards, self.devices, strict=True)
            ]
            put.append(
                jax.make_array_from_single_device_arrays(x.shape, sh, bufs)
            )
        jax.block_until_ready(put)
        return put

    def run(self, concat_in):
        n = self.n_cores
        zeros = [
            np.zeros((n * z.shape[0], *z.shape[1:]), z.dtype)
            for z in self.zero_outs
        ]
        outs = self.sharded(*concat_in, *zeros)
        self.jax.block_until_ready(outs)
        return outs

    def results(self, outs):
        n = self.n_cores
        return [
            {
                name: np.asarray(outs[i]).reshape(n, *self.out_avals[i].shape)[c]
                for i, name in enumerate(self.out_names)
            }
            for c in range(n)
        ]


# ---------------------------------------------------------------------------
# Entry point
# ---------------------------------------------------------------------------


def make_in_maps(plan, x, W1, b1, W2, b2, W3, b3, Wfc, bfc):
    dinv = plan["dinv"]
    xh = np.zeros((N_NODES, ROWP), np.float16)
    xh[:, :IN_F] = (np.asarray(x, np.float32) * dinv[:, None]).astype(np.float16)
    w1p = np.zeros((HID, HID), np.float32)
    w1p[:IN_F, :] = np.asarray(W1, np.float32)
    base = dict(
        xh=xh,
        w1p=w1p,
        w2=np.asarray(W2, np.float32),
        w3=np.asarray(W3, np.float32),
        wfc=np.asarray(Wfc, np.float32),
        b1=np.asarray(b1, np.float32).reshape(HID, 1),
        b2=np.asarray(b2, np.float32).reshape(HID, 1),
        b3=np.asarray(b3, np.float32).reshape(HID, 1),
        ramp32=plan["ramp32"],
        ramp128=plan["ramp128"],
        ident64=np.eye(HID, dtype=np.float16),
    )
    return [
        dict(
            base,
            idx16=plan["idx"][r],
            ncfg=plan["ncfg"][r],
            colidx=plan["colidx"][r],
            pcsub=plan["pcsub"][r],
            dinvrep=plan["dinvrep"][r],
        )
        for r in range(N_CORES)
    ]


_CACHE = {}


def get_runner(plan, reps=1):
    key = ("nc", reps)
    if key not in _CACHE:
        nc = build_nc(plan, reps=reps)
        _CACHE[key] = _Runner(nc, N_CORES)
    return _CACHE[key]


def kernel(x, edge_index, W1, b1, W2, b2, W3, b3, Wfc, bfc):
    plan = build_plan(edge_index)
    runner = get_runner(plan, reps=1)
    in_maps = make_in_maps(plan, x, W1, b1, W2, b2, W3, b3, Wfc, bfc)
    ci = runner.prepare(in_maps)
    res = runner.results(runner.run(ci))
    out = np.concatenate(
        [res[r]["outT"].T for r in range(N_CORES)], axis=0
    ).astype(np.float32)
    return out + np.asarray(bfc, np.float32)[None, :]
